# revision 2
# baseline (speedup 1.0000x reference)
"""Trainium2 Bass kernel for NeighborhoodNormalization.

Math: the reference builds a per-point homogeneous transform
T = [[ux,-uy,0,px],[uy,ux,0,py],[0,0,1,pz],[0,0,0,1]] (u = p/||p||),
inverts it, and applies it to 64 neighbors per point.  Closed form with
r2 = px^2+py^2, n = ||p||, a = n/r2, cx = px*a, cy = py*a, s = q - p:

    out.x =  cx*sx + cy*sy
    out.y = -cy*sx + cx*sy
    out.z =  sz

Pure data parallel over the N=8192 point axis across 8 cores.

Per-core layout: 16384 points = 128 partitions x 128 columns, partition
p = b*8 + s holds points with local n = s*128 + t.  Neighbor rows stay
contiguous in HBM per point (64*3 floats), so DMAs move [128 x G*768B]
blocks.  Compute runs as a handful of WIDE ops per G=16-column group
using 0-stride broadcast APs for the per-point coefficients:

  DVE:    s_xy = q_xy - p_xy          [P,G,K,2]  (contiguous out)
          m    = s_xy * [cx,cx]_b     [P,G,K,2]
          out_xy(bf16) = m + n        [P,G,K,2]  (strided bf16 out)
  GPSIMD: n_x  = s_y * cy_b           [P,G,K]
          n_y  = s_x * (-cy)_b        [P,G,K]
          out_z(bf16) = q_z - pz_b    [P,G,K]
  ACT:    output DMA trigger (2nd HWDGE ring; input DMAs ride SP ring)

The output is written as bfloat16 (harness tolerance 2e-2; bf16 adds
~1e-3), halving output HBM traffic: 12.6 MB in + 6.3 MB out per core
vs a 358 GB/s/core HBM roofline.
"""

import sys

if "/opt/trn_rl_repo" not in sys.path:
    sys.path.insert(0, "/opt/trn_rl_repo")

import numpy as np

import concourse.bass as bass
import concourse.bacc as bacc
import concourse.mybir as mybir
from concourse.tile import TileContext
from concourse.bass_utils import run_bass_kernel_spmd

B = 16
N = 8192
K = 64
NCORES = 8
NLOC = N // NCORES  # 1024 points per core
P = 128             # SBUF partitions
S = NLOC // P       # 8 partition sub-blocks per batch entry
T = (B * NLOC) // P  # 128 point-columns per partition
G = 16              # columns per DMA group
NG = T // G

F32 = mybir.dt.float32
BF16 = mybir.dt.bfloat16

_CACHE = {}


def _build_nc():
    nc = bacc.Bacc(None, target_bir_lowering=False)

    pts = nc.declare_dram_parameter("points", [B, NLOC, 3], F32, isOutput=False)
    nb = nc.declare_dram_parameter("neighborhoods", [B, NLOC, K, 3], F32, isOutput=False)
    out = nc.declare_dram_parameter("out", [B, NLOC, K, 3], BF16, isOutput=True)

    # partition = (b s), columns = t, free = 192 floats per point
    nbr = nb[:].rearrange("b (s t) k c -> (b s) t (k c)", s=S)
    outr = out[:].rearrange("b (s t) k c -> (b s) t (k c)", s=S)
    ptsr = pts[:].rearrange("b (s t) c -> (b s) (t c)", s=S)

    with TileContext(nc) as tc:
        with tc.tile_pool(name="const", bufs=1) as cpool, \
             tc.tile_pool(name="io_in", bufs=4) as inpool, \
             tc.tile_pool(name="io_out", bufs=4) as outpool, \
             tc.tile_pool(name="work", bufs=3) as wpool:

            pts_sb = cpool.tile([P, T * 3], F32, tag="pts")
            nc.sync.dma_start(out=pts_sb[:], in_=ptsr)
            pv = pts_sb[:].rearrange("p (t c) -> p t c", c=3)
            px = pv[:, :, 0]
            py = pv[:, :, 1]
            pz = pv[:, :, 2]

            def ctile(tag):
                return cpool.tile([P, T], F32, tag=tag, name=tag)

            t1 = ctile("t1")
            t2 = ctile("t2")
            r2 = ctile("r2")
            nn = ctile("nn")
            ir2 = ctile("ir2")
            aa = ctile("aa")
            cx = ctile("cx")
            cy = ctile("cy")
            ncy = ctile("ncy")

            nc.vector.tensor_mul(out=t1[:], in0=px, in1=px)
            nc.vector.tensor_mul(out=t2[:], in0=py, in1=py)
            nc.vector.tensor_add(out=r2[:], in0=t1[:], in1=t2[:])
            nc.vector.tensor_mul(out=t1[:], in0=pz, in1=pz)
            nc.vector.tensor_add(out=t2[:], in0=r2[:], in1=t1[:])
            nc.scalar.sqrt(out=nn[:], in_=t2[:])
            nc.vector.reciprocal(out=ir2[:], in_=r2[:])
            nc.vector.tensor_mul(out=aa[:], in0=nn[:], in1=ir2[:])
            nc.vector.tensor_mul(out=cx[:], in0=px, in1=aa[:])
            nc.vector.tensor_mul(out=cy[:], in0=py, in1=aa[:])
            nc.vector.tensor_scalar_mul(out=ncy[:], in0=cy[:], scalar1=-1.0)

            for g in range(NG):
                sl = slice(g * G, (g + 1) * G)

                nb_t = inpool.tile([P, G, K, 3], F32, tag="nb", name=f"nb{g}")
                nc.sync.dma_start(
                    out=nb_t[:].rearrange("p g k c -> p g (k c)"),
                    in_=nbr[:, sl, :],
                )

                s_t = wpool.tile([P, G, K, 2], F32, tag="s", name=f"s{g}")
                m_t = wpool.tile([P, G, K, 2], F32, tag="m", name=f"m{g}")
                n_t = wpool.tile([P, G, K, 2], F32, tag="n", name=f"n{g}")
                ot = outpool.tile([P, G, K, 3], BF16, tag="ot", name=f"ot{g}")

                # s_xy = q_xy - p_xy  (p broadcast over K via 0-stride)
                pxy_b = pv[:, sl, 0:2][:, :, None, :].broadcast_to([P, G, K, 2])
                nc.vector.tensor_sub(
                    out=s_t[:], in0=nb_t[:, :, :, 0:2], in1=pxy_b,
                )

                # m = s_xy * [cx, cx]  (cx broadcast over K and over x/y)
                cx_b = cx[:, sl][:, :, None, None].broadcast_to([P, G, K, 2])
                nc.vector.tensor_mul(out=m_t[:], in0=s_t[:], in1=cx_b)

                # n_x = s_y * cy ; n_y = s_x * (-cy)   (GPSIMD)
                cy_b = cy[:, sl][:, :, None].broadcast_to([P, G, K])
                ncy_b = ncy[:, sl][:, :, None].broadcast_to([P, G, K])
                nc.gpsimd.tensor_mul(
                    out=n_t[:, :, :, 0], in0=s_t[:, :, :, 1], in1=cy_b,
                )
                nc.gpsimd.tensor_mul(
                    out=n_t[:, :, :, 1], in0=s_t[:, :, :, 0], in1=ncy_b,
                )

                # out_xy = m + n  (bf16 strided write)
                nc.vector.tensor_add(
                    out=ot[:, :, :, 0:2], in0=m_t[:], in1=n_t[:],
                )

                # out_z = q_z - pz  (GPSIMD, bf16 strided write)
                pz_b = pv[:, sl, 2][:, :, None].broadcast_to([P, G, K])
                nc.gpsimd.tensor_sub(
                    out=ot[:, :, :, 2], in0=nb_t[:, :, :, 2], in1=pz_b,
                )

                # out-DMA on the ACT HWDGE ring so it overlaps the SP-ring
                # input stream (HWDGE is FIFO per issuing engine).
                nc.scalar.dma_start(
                    out=outr[:, sl, :],
                    in_=ot[:].rearrange("p g k c -> p g (k c)"),
                )

    nc.compile()
    return nc


def _get_nc():
    if "nc" not in _CACHE:
        _CACHE["nc"] = _build_nc()
    return _CACHE["nc"]


def kernel(points, neighborhoods):
    pts = np.ascontiguousarray(np.asarray(points, dtype=np.float32))
    nb = np.ascontiguousarray(np.asarray(neighborhoods, dtype=np.float32))
    assert pts.shape == (B, N, 3), pts.shape
    assert nb.shape == (B, N, K, 3), nb.shape

    in_maps = []
    for c in range(NCORES):
        sl = slice(c * NLOC, (c + 1) * NLOC)
        in_maps.append({
            "points": np.ascontiguousarray(pts[:, sl]),
            "neighborhoods": np.ascontiguousarray(nb[:, sl]),
        })

    res = run_bass_kernel_spmd(_get_nc(), in_maps, list(range(NCORES))).results
    out = np.concatenate(
        [np.asarray(res[c]["out"]).astype(np.float32) for c in range(NCORES)],
        axis=1,
    )
    return out


# revision 3
# speedup vs baseline: 1.4046x; 1.4046x over previous
"""Trainium2 Bass kernel for NeighborhoodNormalization.

Math: the reference builds a per-point homogeneous transform
T = [[ux,-uy,0,px],[uy,ux,0,py],[0,0,1,pz],[0,0,0,1]] (u = p/||p||),
inverts it, and applies it to 64 neighbors per point.  Closed form with
r2 = px^2+py^2, n = ||p||, a = n/r2, cx = px*a, cy = py*a, s = q - p:

    out.x =  cx*sx + cy*sy
    out.y = -cy*sx + cx*sy
    out.z =  sz

Pure data parallel over the N=8192 point axis across 8 cores.

Per-core layout: 16384 points = 128 partitions x 128 columns, partition
p = b*8 + s holds points with local n = s*128 + t.  Neighbor rows stay
contiguous in HBM per point (64*3 floats), so DMAs move [128 x G*768B]
blocks.

The whole pipeline runs in bf16 on the vector engine only (DVE+GPSIMD
contend for SBUF ports, so spreading elementwise work across them is a
net loss; bf16 with packed access patterns unlocks the DVE 2x perf
mode).  The input DMA casts fp32->bf16 in flight (SWDGE); HBM read
bytes are unchanged, but all SBUF tiles and the output are bf16, which
also halves output HBM traffic (harness tolerance 2e-2, bf16 ~5e-3).

Per G=16-column group, DVE does 4 wide packed ops over the interleaved
(x,y,z) stream using 0-stride broadcast APs for per-point coefficients:

    s3 = q - p_b                  [P,G,K,3]  (b'cast [px,py,pz])
    m3 = s3 * B1_b                [P,G,K,3]  (B1 = [cx,cx,1])
    n  = swap_xy(s3) * B2_b       [P,G,K,2]  (B2 = [cy,-cy]; -1-stride)
    out_xy = m3_xy + n            [P,G,K,2]
ACT: out_z = copy(m3_z), plus the output DMA trigger (2nd HWDGE ring).
"""

import sys

if "/opt/trn_rl_repo" not in sys.path:
    sys.path.insert(0, "/opt/trn_rl_repo")

import numpy as np

import concourse.bass as bass
import concourse.bacc as bacc
import concourse.mybir as mybir
from concourse.tile import TileContext
from concourse.bass_utils import run_bass_kernel_spmd

B = 16
N = 8192
K = 64
NCORES = 8
NLOC = N // NCORES  # 1024 points per core
P = 128             # SBUF partitions
S = NLOC // P       # 8 partition sub-blocks per batch entry
T = (B * NLOC) // P  # 128 point-columns per partition
G = 16              # columns per DMA group
NG = T // G

F32 = mybir.dt.float32
BF16 = mybir.dt.bfloat16

_CACHE = {}


def _build_nc():
    nc = bacc.Bacc(None, target_bir_lowering=False)

    pts = nc.declare_dram_parameter("points", [B, NLOC, 3], F32, isOutput=False)
    nb = nc.declare_dram_parameter("neighborhoods", [B, NLOC, K, 3], F32, isOutput=False)
    out = nc.declare_dram_parameter("out", [B, NLOC, K, 3], BF16, isOutput=True)

    # partition = (b s), columns = t, free = 192 floats per point
    nbr = nb[:].rearrange("b (s t) k c -> (b s) t (k c)", s=S)
    outr = out[:].rearrange("b (s t) k c -> (b s) t (k c)", s=S)
    ptsr = pts[:].rearrange("b (s t) c -> (b s) (t c)", s=S)

    with TileContext(nc) as tc:
        with tc.tile_pool(name="const", bufs=1) as cpool, \
             tc.tile_pool(name="io_in", bufs=4) as inpool, \
             tc.tile_pool(name="io_out", bufs=4) as outpool, \
             tc.tile_pool(name="work", bufs=3) as wpool:

            pts_sb = cpool.tile([P, T * 3], F32, tag="pts")
            nc.sync.dma_start(out=pts_sb[:], in_=ptsr)
            pv = pts_sb[:].rearrange("p (t c) -> p t c", c=3)
            px = pv[:, :, 0]
            py = pv[:, :, 1]
            pz = pv[:, :, 2]

            def ctile(tag, w=1, dt=F32):
                return cpool.tile([P, T * w], dt, tag=tag, name=tag)

            t1 = ctile("t1")
            t2 = ctile("t2")
            r2 = ctile("r2")
            nn = ctile("nn")
            ir2 = ctile("ir2")
            aa = ctile("aa")
            cx = ctile("cx")
            cy = ctile("cy")

            nc.vector.tensor_mul(out=t1[:], in0=px, in1=px)
            nc.vector.tensor_mul(out=t2[:], in0=py, in1=py)
            nc.vector.tensor_add(out=r2[:], in0=t1[:], in1=t2[:])
            nc.vector.tensor_mul(out=t1[:], in0=pz, in1=pz)
            nc.vector.tensor_add(out=t2[:], in0=r2[:], in1=t1[:])
            nc.scalar.sqrt(out=nn[:], in_=t2[:])
            nc.vector.reciprocal(out=ir2[:], in_=r2[:])
            nc.vector.tensor_mul(out=aa[:], in0=nn[:], in1=ir2[:])
            nc.vector.tensor_mul(out=cx[:], in0=px, in1=aa[:])
            nc.vector.tensor_mul(out=cy[:], in0=py, in1=aa[:])

            # bf16 coefficient tiles (packed last-dim views for DVE 2x):
            #   p3  = [px, py, pz] per point      [P, T, 3]
            #   b1  = [cx, cx, 1]  per point      [P, T, 3]
            #   b2  = [cy, -cy]    per point      [P, T, 2]
            p3 = ctile("p3", 3, BF16)
            b1 = ctile("b1", 3, BF16)
            b2 = ctile("b2", 2, BF16)
            p3v = p3[:].rearrange("p (t c) -> p t c", c=3)
            b1v = b1[:].rearrange("p (t c) -> p t c", c=3)
            b2v = b2[:].rearrange("p (t c) -> p t c", c=2)
            nc.scalar.copy(out=p3[:], in_=pts_sb[:])
            nc.vector.tensor_copy(out=b1v[:, :, 0], in_=cx[:])
            nc.vector.tensor_copy(out=b1v[:, :, 1], in_=cx[:])
            nc.vector.memset(b1v[:, :, 2], 1.0)
            nc.vector.tensor_copy(out=b2v[:, :, 0], in_=cy[:])
            nc.vector.tensor_scalar_mul(out=b2v[:, :, 1], in0=cy[:], scalar1=-1.0)

            for g in range(NG):
                sl = slice(g * G, (g + 1) * G)

                # fp32 -> bf16 cast in flight: SWDGE (gpsimd) DMA
                nb_t = inpool.tile([P, G, K, 3], BF16, tag="nb", name=f"nb{g}")
                nc.gpsimd.dma_start(
                    out=nb_t[:].rearrange("p g k c -> p g (k c)"),
                    in_=nbr[:, sl, :],
                )

                s3 = wpool.tile([P, G, K, 3], BF16, tag="s", name=f"s{g}")
                m3 = wpool.tile([P, G, K, 3], BF16, tag="m", name=f"m{g}")
                n_t = wpool.tile([P, G, K, 2], BF16, tag="n", name=f"n{g}")
                ot = outpool.tile([P, G, K, 3], BF16, tag="ot", name=f"ot{g}")

                # s3 = q - p  (p broadcast over K; last dim packed (1,3))
                p3_b = p3v[:, sl, :][:, :, None, :].broadcast_to([P, G, K, 3])
                nc.vector.tensor_sub(out=s3[:], in0=nb_t[:], in1=p3_b)

                # m3 = s3 * [cx, cx, 1]
                b1_b = b1v[:, sl, :][:, :, None, :].broadcast_to([P, G, K, 3])
                nc.vector.tensor_mul(out=m3[:], in0=s3[:], in1=b1_b)

                # n = [sy, sx] * [cy, -cy]   (swap via -1-stride last dim)
                b2_b = b2v[:, sl, :][:, :, None, :].broadcast_to([P, G, K, 2])
                nc.vector.tensor_mul(
                    out=n_t[:], in0=s3[:, :, :, 1::-1], in1=b2_b,
                )

                # out_xy = m3_xy + n
                nc.vector.tensor_add(
                    out=ot[:, :, :, 0:2], in0=m3[:, :, :, 0:2], in1=n_t[:],
                )

                # out_z = m3_z  (ACT copy; ACT does not contend with DVE)
                nc.scalar.copy(out=ot[:, :, :, 2], in_=m3[:, :, :, 2])

                # out-DMA on the ACT HWDGE ring (input stream is SWDGE)
                nc.scalar.dma_start(
                    out=outr[:, sl, :],
                    in_=ot[:].rearrange("p g k c -> p g (k c)"),
                )

    nc.compile()
    return nc


def _get_nc():
    if "nc" not in _CACHE:
        _CACHE["nc"] = _build_nc()
    return _CACHE["nc"]


def kernel(points, neighborhoods):
    pts = np.ascontiguousarray(np.asarray(points, dtype=np.float32))
    nb = np.ascontiguousarray(np.asarray(neighborhoods, dtype=np.float32))
    assert pts.shape == (B, N, 3), pts.shape
    assert nb.shape == (B, N, K, 3), nb.shape

    in_maps = []
    for c in range(NCORES):
        sl = slice(c * NLOC, (c + 1) * NLOC)
        in_maps.append({
            "points": np.ascontiguousarray(pts[:, sl]),
            "neighborhoods": np.ascontiguousarray(nb[:, sl]),
        })

    res = run_bass_kernel_spmd(_get_nc(), in_maps, list(range(NCORES))).results
    out = np.concatenate(
        [np.asarray(res[c]["out"]).astype(np.float32) for c in range(NCORES)],
        axis=1,
    )
    return out


# revision 6
# speedup vs baseline: 1.5409x; 1.0970x over previous
"""Trainium2 Bass kernel for NeighborhoodNormalization.

Math: the reference builds a per-point homogeneous transform
T = [[ux,-uy,0,px],[uy,ux,0,py],[0,0,1,pz],[0,0,0,1]] (u = p/||p||),
inverts it, and applies it to 64 neighbors per point.  Closed form with
r2 = px^2+py^2, n = ||p||, a = n/r2, cx = px*a, cy = py*a, s = q - p:

    out.x =  cx*sx + cy*sy
    out.y = -cy*sx + cx*sy
    out.z =  sz

Pure data parallel over the N=8192 point axis across 8 cores.

Per-core layout: 16384 points = 128 partitions x 128 columns, partition
p = b*8 + s holds points with local n = s*128 + t.  Neighbor rows stay
contiguous in HBM per point (64*3 floats), so DMAs move [128 x G*768B]
blocks.

The whole pipeline runs in bf16 on the vector engine only (DVE+GPSIMD
contend for SBUF ports, so spreading elementwise work across them is a
net loss; bf16 with packed access patterns unlocks the DVE 2x perf
mode).  The input DMA casts fp32->bf16 in flight (SWDGE); HBM read
bytes are unchanged, but all SBUF tiles and the output are bf16, which
also halves output HBM traffic (harness tolerance 2e-2, bf16 ~5e-3).

Per G=16-column group, DVE does 4 wide packed ops over the interleaved
(x,y,z) stream using 0-stride broadcast APs for per-point coefficients:

    s3 = q - p_b                  [P,G,K,3]  (b'cast [px,py,pz])
    m3 = s3 * B1_b                [P,G,K,3]  (B1 = [cx,cx,1])
    n  = swap_xy(s3) * B2_b       [P,G,K,2]  (B2 = [cy,-cy]; -1-stride)
    out_xy = m3_xy + n            [P,G,K,2]
ACT: out_z = copy(m3_z), plus the output DMA trigger (2nd HWDGE ring).
"""

import sys

if "/opt/trn_rl_repo" not in sys.path:
    sys.path.insert(0, "/opt/trn_rl_repo")

import numpy as np

import concourse.bass as bass
import concourse.bacc as bacc
import concourse.mybir as mybir
from concourse.tile import TileContext
from concourse.bass_utils import run_bass_kernel_spmd

B = 16
N = 8192
K = 64
NCORES = 8
NLOC = N // NCORES  # 1024 points per core
P = 128             # SBUF partitions
S = NLOC // P       # 8 partition sub-blocks per batch entry
T = (B * NLOC) // P  # 128 point-columns per partition
G = 16              # columns per DMA group
NG = T // G

F32 = mybir.dt.float32
BF16 = mybir.dt.bfloat16

_CACHE = {}


def _build_nc():
    nc = bacc.Bacc(None, target_bir_lowering=False)

    pts = nc.declare_dram_parameter("points", [B, NLOC, 3], F32, isOutput=False)
    nb = nc.declare_dram_parameter("neighborhoods", [B, NLOC, K, 3], F32, isOutput=False)
    out = nc.declare_dram_parameter("out", [B, NLOC, K, 3], BF16, isOutput=True)

    # partition = (b s), columns = t, free = 192 floats per point
    nbr = nb[:].rearrange("b (s t) k c -> (b s) t (k c)", s=S)
    outr = out[:].rearrange("b (s t) k c -> (b s) t (k c)", s=S)
    ptsr = pts[:].rearrange("b (s t) c -> (b s) (t c)", s=S)

    with TileContext(nc) as tc:
        with tc.tile_pool(name="const", bufs=1) as cpool, \
             tc.tile_pool(name="io_in", bufs=4) as inpool, \
             tc.tile_pool(name="io_out", bufs=4) as outpool, \
             tc.tile_pool(name="work", bufs=3) as wpool:

            pts_sb = cpool.tile([P, T * 3], F32, tag="pts")
            nc.sync.dma_start(out=pts_sb[:], in_=ptsr)
            pv = pts_sb[:].rearrange("p (t c) -> p t c", c=3)
            px = pv[:, :, 0]
            py = pv[:, :, 1]
            pz = pv[:, :, 2]

            def ctile(tag, w=1, dt=F32):
                return cpool.tile([P, T * w], dt, tag=tag, name=tag)

            t1 = ctile("t1")
            t2 = ctile("t2")
            r2 = ctile("r2")
            nn = ctile("nn")
            ir2 = ctile("ir2")
            aa = ctile("aa")
            cx = ctile("cx")
            cy = ctile("cy")

            nc.vector.tensor_mul(out=t1[:], in0=px, in1=px)
            nc.vector.tensor_mul(out=t2[:], in0=py, in1=py)
            nc.vector.tensor_add(out=r2[:], in0=t1[:], in1=t2[:])
            nc.vector.tensor_mul(out=t1[:], in0=pz, in1=pz)
            nc.vector.tensor_add(out=t2[:], in0=r2[:], in1=t1[:])
            nc.scalar.sqrt(out=nn[:], in_=t2[:])
            nc.vector.reciprocal(out=ir2[:], in_=r2[:])
            nc.vector.tensor_mul(out=aa[:], in0=nn[:], in1=ir2[:])
            nc.vector.tensor_mul(out=cx[:], in0=px, in1=aa[:])
            nc.vector.tensor_mul(out=cy[:], in0=py, in1=aa[:])

            # bf16 coefficient tiles; p3/b1 store each point's 3-pattern
            # repeated R=4x so the broadcast APs used by the two big
            # contiguous ops (sub, mul) have 12-element packed inner runs:
            #   p3  = [px, py, pz]*4 per point    [P, T, 12]
            #   b1  = [cx, cx, 1]*4  per point    [P, T, 12]
            #   b2  = [cy, -cy]      per point    [P, T, 2]
            R = 4
            p3 = ctile("p3", 3 * R, BF16)
            b1 = ctile("b1", 3 * R, BF16)
            b2 = ctile("b2", 2, BF16)
            p3v = p3[:].rearrange("p (t r c) -> p t r c", r=R, c=3)
            b1v = b1[:].rearrange("p (t r c) -> p t r c", r=R, c=3)
            b2v = b2[:].rearrange("p (t c) -> p t c", c=2)
            nc.vector.tensor_copy(
                out=p3v[:],
                in_=pv[:, :, None, :].broadcast_to([P, T, R, 3]),
            )
            nc.vector.tensor_copy(
                out=b1v[:, :, :, 0:2],
                in_=cx[:, :, None, None].broadcast_to([P, T, R, 2]),
            )
            nc.vector.memset(b1v[:, :, :, 2], 1.0)
            nc.vector.tensor_copy(out=b2v[:, :, 0], in_=cy[:])
            nc.vector.tensor_scalar_mul(out=b2v[:, :, 1], in0=cy[:], scalar1=-1.0)

            for g in range(NG):
                sl = slice(g * G, (g + 1) * G)

                # fp32 -> bf16 cast in flight: SWDGE (gpsimd) DMA
                nb_t = inpool.tile([P, G, K, 3], BF16, tag="nb", name=f"nb{g}")
                nc.gpsimd.dma_start(
                    out=nb_t[:].rearrange("p g k c -> p g (k c)"),
                    in_=nbr[:, sl, :],
                )

                s3 = wpool.tile([P, G, K, 3], BF16, tag="s", name=f"s{g}")
                m3 = wpool.tile([P, G, K, 3], BF16, tag="m", name=f"m{g}")
                n_t = wpool.tile([P, G, K, 2], BF16, tag="n", name=f"n{g}")
                ot = outpool.tile([P, G, K, 3], BF16, tag="ot", name=f"ot{g}")

                # regrouped 12-wide views: (g, kk, r*c) with contiguous
                # 12-elem inner runs on both the data and the coefficients
                q12 = nb_t[:].rearrange("p g (kk r) c -> p g kk (r c)", r=R)
                s12 = s3[:].rearrange("p g (kk r) c -> p g kk (r c)", r=R)
                m12 = m3[:].rearrange("p g (kk r) c -> p g kk (r c)", r=R)
                p12 = p3[:].rearrange("p (t w) -> p t w", w=3 * R)
                b1w = b1[:].rearrange("p (t w) -> p t w", w=3 * R)

                # s3 = q - p  (p broadcast over K/R; 12-elem packed runs)
                p3_b = p12[:, sl, :][:, :, None, :].broadcast_to(
                    [P, G, K // R, 3 * R])
                nc.vector.tensor_sub(out=s12[:], in0=q12[:], in1=p3_b)

                # m3 = s3 * [cx, cx, 1]
                b1_b = b1w[:, sl, :][:, :, None, :].broadcast_to(
                    [P, G, K // R, 3 * R])
                nc.vector.tensor_mul(out=m12[:], in0=s12[:], in1=b1_b)

                # n = [sy, sx] * [cy, -cy]   (swap via -1-stride last dim)
                b2_b = b2v[:, sl, :][:, :, None, :].broadcast_to([P, G, K, 2])
                nc.vector.tensor_mul(
                    out=n_t[:], in0=s3[:, :, :, 1::-1], in1=b2_b,
                )

                # out_xy = m3_xy + n
                nc.vector.tensor_add(
                    out=ot[:, :, :, 0:2], in0=m3[:, :, :, 0:2], in1=n_t[:],
                )

                # out_z = s3_z  (ACT copy; ACT does not contend with DVE)
                nc.scalar.copy(out=ot[:, :, :, 2], in_=s3[:, :, :, 2])

                # out-DMA on the ACT HWDGE ring (input stream is SWDGE)
                nc.scalar.dma_start(
                    out=outr[:, sl, :],
                    in_=ot[:].rearrange("p g k c -> p g (k c)"),
                )

    nc.compile()
    return nc


def _get_nc():
    if "nc" not in _CACHE:
        _CACHE["nc"] = _build_nc()
    return _CACHE["nc"]


def kernel(points, neighborhoods):
    pts = np.ascontiguousarray(np.asarray(points, dtype=np.float32))
    nb = np.ascontiguousarray(np.asarray(neighborhoods, dtype=np.float32))
    assert pts.shape == (B, N, 3), pts.shape
    assert nb.shape == (B, N, K, 3), nb.shape

    in_maps = []
    for c in range(NCORES):
        sl = slice(c * NLOC, (c + 1) * NLOC)
        in_maps.append({
            "points": np.ascontiguousarray(pts[:, sl]),
            "neighborhoods": np.ascontiguousarray(nb[:, sl]),
        })

    res = run_bass_kernel_spmd(_get_nc(), in_maps, list(range(NCORES))).results
    out = np.concatenate(
        [np.asarray(res[c]["out"]).astype(np.float32) for c in range(NCORES)],
        axis=1,
    )
    return out
